# revision 1
# baseline (speedup 1.0000x reference)
"""GEAR quantized-KV Llama attention decode step on 8 trn2 NeuronCores.

Sharding: tensor-parallel over heads (4 heads/core x 8 cores), all batches on
every core; each core computes a partial wo-product, summed on host.
"""
import os
import sys
import math

sys.path.insert(0, "/opt/trn_rl_repo")
import numpy as np
from contextlib import ExitStack

import concourse.bass as bass
import concourse.mybir as mybir
import concourse.tile as tile
from concourse import bacc, bass_isa
from concourse.bass_utils import run_bass_kernel_spmd
from concourse.masks import make_identity

B, H, D, HID = 4, 32, 128, 4096
SQ, SF, QL = 4096, 63, 1
GS, RANK = 64, 4
THETA = 10000.0
NCORES = 8
HPC = H // NCORES          # heads per core = 4
NCH = SQ // 128            # 32 s-chunks
G = SQ // GS               # 64 groups along seq (K side)
FD = D // GS               # 2 groups along head_dim (V side)
SFP = SF + 1               # 64 full-precision keys incl the new token
DT = mybir.dt
ISQD = 1.0 / math.sqrt(D)

_CACHE = {}


def _build():
    nc = bacc.Bacc("TRN2", target_bir_lowering=False)
    f32, bf16, i32 = DT.float32, DT.bfloat16, DT.int32

    hidT = nc.declare_dram_parameter("hidT", [HID, B], f32, isOutput=False)
    cost = nc.declare_dram_parameter("cost", [B, HPC * D], f32, isOutput=False)
    sint = nc.declare_dram_parameter("sint", [B, HPC * D], f32, isOutput=False)
    wT = {w: nc.declare_dram_parameter(w, [HID, HPC * D], f32, isOutput=False) for w in ("wqT", "wkT", "wvT")}
    woT = nc.declare_dram_parameter("woT", [HPC * D, HID], f32, isOutput=False)
    kcode = nc.declare_dram_parameter("kcode", [B, HPC, D, SQ], i32, isOutput=False)
    kscale = nc.declare_dram_parameter("kscale", [B, HPC, D, G], f32, isOutput=False)
    kmn = nc.declare_dram_parameter("kmn", [B, HPC, D, G], f32, isOutput=False)
    kfT = nc.declare_dram_parameter("kfT", [B, HPC, D, SF], f32, isOutput=False)
    kp = nc.declare_dram_parameter("kp", [B, HPC, 128, NCH, RANK], f32, isOutput=False)
    keyq = nc.declare_dram_parameter("keyq", [B, HPC, D, RANK], f32, isOutput=False)
    vcode = nc.declare_dram_parameter("vcode", [B, HPC, SQ, D], i32, isOutput=False)
    vscT = nc.declare_dram_parameter("vscT", [B, HPC, 128, NCH, FD], f32, isOutput=False)
    vmnT = nc.declare_dram_parameter("vmnT", [B, HPC, 128, NCH, FD], f32, isOutput=False)
    vqT = nc.declare_dram_parameter("vqT", [B, HPC, 128, NCH, RANK], f32, isOutput=False)
    vpT = nc.declare_dram_parameter("vpT", [B, HPC, 7, D], f32, isOutput=False)  # rows 0-2 zero
    vfr = nc.declare_dram_parameter("vfr", [B, HPC, SF, D], f32, isOutput=False)
    out = nc.declare_dram_parameter("out", [B, HID], f32, isOutput=True)

    AO = mybir.AluOpType
    AF = mybir.ActivationFunctionType

    with tile.TileContext(nc) as tc, ExitStack() as ctx:
        const = ctx.enter_context(tc.tile_pool(name="const", bufs=1))
        pw = ctx.enter_context(tc.tile_pool(name="pw", bufs=2))
        psC = ctx.enter_context(tc.tile_pool(name="psC", bufs=2, space="PSUM"))
        psW = ctx.enter_context(tc.tile_pool(name="psW", bufs=1, space="PSUM"))
        ictx = ctx.enter_context(ExitStack())
        psml = ictx.enter_context(tc.tile_pool(name="psml", bufs=3))
        pkc = ictx.enter_context(tc.tile_pool(name="pkc", bufs=2))
        pvt = ictx.enter_context(tc.tile_pool(name="pvt", bufs=2))
        psA = ictx.enter_context(tc.tile_pool(name="psA", bufs=2, space="PSUM"))
        psB = ictx.enter_context(tc.tile_pool(name="psB", bufs=2, space="PSUM"))

        # ---- constants ----
        id4 = const.tile([4, 4], f32)
        make_identity(nc, id4[:])
        id16 = const.tile([16, 16], f32)
        make_identity(nc, id16[:], nomemset=False)
        hid_sb = const.tile([128, HID // 128, B], f32)
        nc.sync.dma_start(out=hid_sb[:], in_=hidT[:].rearrange("(c p) b -> p c b", p=128))
        cos_sb = const.tile([B, HPC * D], f32)
        nc.sync.dma_start(out=cos_sb[:], in_=cost[:])
        sin_sb = const.tile([B, HPC * D], f32)
        nc.sync.dma_start(out=sin_sb[:], in_=sint[:])

        # ---- projections: psum[b, 512] = sum_c hidT_c^T @ wT_c ----
        proj = {}
        for wname in ("wqT", "wkT", "wvT"):
            pps = psC.tile([B, HPC * D], f32, tag="misc")
            for blk in range(4):
                slab = pw.tile([128, 8, HPC * D], f32, tag="wslab")
                nc.sync.dma_start(
                    out=slab[:],
                    in_=wT[wname][:].rearrange("(c p) n -> p c n", p=128)[:, 8 * blk:8 * blk + 8, :],
                )
                for j in range(8):
                    c = 8 * blk + j
                    nc.tensor.matmul(pps[:], hid_sb[:, c, :], slab[:, j, :],
                                     start=(c == 0), stop=(c == 31))
            sb = const.tile([B, HPC * D], f32, tag=wname)
            nc.scalar.copy(sb[:], pps[:])
            proj[wname] = sb
        q_sb, k_sb, v_sb = proj["wqT"], proj["wkT"], proj["wvT"]

        # ---- RoPE on q and k (rows [B, HPC*D]) ----
        def rope(x_sb, tagp):
            rot = const.tile([B, HPC * D], f32, tag=tagp + "rot")
            xv = x_sb[:].rearrange("b (h two d) -> b h two d", two=2, d=64)
            rv = rot[:].rearrange("b (h two d) -> b h two d", two=2, d=64)
            nc.vector.tensor_scalar(rv[:, :, 0, :], xv[:, :, 1, :], -1.0, None, AO.mult)
            nc.vector.tensor_copy(rv[:, :, 1, :], xv[:, :, 0, :])
            nc.vector.tensor_tensor(rot[:], rot[:], sin_sb[:], AO.mult)
            ro = const.tile([B, HPC * D], f32, tag=tagp + "ro")
            nc.vector.tensor_tensor(ro[:], x_sb[:], cos_sb[:], AO.mult)
            nc.vector.tensor_tensor(ro[:], ro[:], rot[:], AO.add)
            return ro
        qro = rope(q_sb, "q")
        kro = rope(k_sb, "k")

        # per-head transposed columns: qscT [128, h, b] (scaled by 1/sqrt(D)), kT
        qscT = const.tile([128, HPC, B], f32)
        kT = const.tile([128, HPC, B], f32)
        for h in range(HPC):
            pq = psC.tile([128, B], f32, tag="misc")
            nc.tensor.transpose(pq[:], qro[0:B, h * D:(h + 1) * D], id4[:])
            nc.scalar.mul(qscT[:, h, :], pq[:], ISQD)
            pk = psC.tile([128, B], f32, tag="misc")
            nc.tensor.transpose(pk[:], kro[0:B, h * D:(h + 1) * D], id4[:])
            nc.scalar.copy(kT[:, h, :], pk[:])

        rows_sb = const.tile([16, 128], f32)
        woin_ps = psW.tile([128, 16], f32)

        # ---- per (b, h) attention ----
        for b in range(B):
            for h in range(HPC):
                idx = h * B + b
                qcol = qscT[:, h, b:b + 1]

                kc_bf = pkc.tile([128, SQ], bf16, tag="kc")
                nc.gpsimd.dma_start(out=kc_bf[:], in_=kcode[b, h])
                ksc = psml.tile([128, G], f32, tag="ksc")
                nc.sync.dma_start(out=ksc[:], in_=kscale[b, h])
                kmn_sb = psml.tile([128, G], f32, tag="kmn")
                nc.sync.dma_start(out=kmn_sb[:], in_=kmn[b, h])
                kfp = psml.tile([128, SFP], f32, tag="kfp")
                nc.sync.dma_start(out=kfp[:, 0:SF], in_=kfT[b, h])
                kp_sb = psml.tile([128, NCH, RANK], f32, tag="kp")
                nc.sync.dma_start(out=kp_sb[:], in_=kp[b, h])
                keyq_sb = psml.tile([128, RANK], f32, tag="keyq")
                nc.sync.dma_start(out=keyq_sb[:], in_=keyq[b, h])
                vt = pvt.tile([128, NCH, 131], bf16, tag="vt")
                nc.gpsimd.dma_start(out=vt[:, :, 0:128],
                                    in_=vcode[b, h].rearrange("(c p) d -> p c d", p=128))
                nc.gpsimd.dma_start(out=vt[:, :, 128:130], in_=vmnT[b, h])
                aw3 = psml.tile([128, NCH, 7], bf16, tag="aw3")
                nc.gpsimd.dma_start(out=aw3[:, :, 3:7], in_=vqT[b, h])
                vsc = psml.tile([128, NCH, FD], f32, tag="vsc")
                nc.sync.dma_start(out=vsc[:], in_=vscT[b, h])
                vpT_sb = psml.tile([7, D], f32, tag="vpT")
                nc.sync.dma_start(out=vpT_sb[:], in_=vpT[b, h])
                vf_sb = psml.tile([SFP, D], f32, tag="vf")
                nc.sync.dma_start(out=vf_sb[0:SF, :], in_=vfr[b, h])
                # new-token k/v into the full-precision blocks
                nc.vector.tensor_copy(kfp[:, SF:SFP], kT[:, h, b:b + 1])
                nc.sync.dma_start(out=vf_sb[SF:SFP, :], in_=v_sb[b:b + 1, h * D:(h + 1) * D])

                # quant K scores: psk[s, 2c + g'] over chunks
                qs = psml.tile([128, G], bf16, tag="qs")
                nc.vector.tensor_scalar(qs[:], ksc[:], qcol, None, AO.mult)
                psk = psA.tile([128, 2 * NCH], f32, tag="psk")
                for c in range(NCH):
                    nc.tensor.matmul(psk[:, 2 * c:2 * c + 2], kc_bf[:, c * 128:(c + 1) * 128],
                                     qs[:, 2 * c:2 * c + 2], start=True, stop=True)
                # misc: kf scores [0:64, 0:1]; qr row [0:1, 32:36]; mn bias row [0:1, 64:128]
                psm = psC.tile([128, 128], f32, tag="misc")
                nc.tensor.matmul(psm[0:SFP, 0:1], kfp[:], qcol, start=True, stop=True)
                nc.tensor.matmul(psm[0:1, 32:36], qcol, keyq_sb[:], start=True, stop=True)
                nc.tensor.matmul(psm[0:1, 64:128], qcol, kmn_sb[:], start=True, stop=True)

                qr_sb = psml.tile([1, RANK], f32, tag="qr")
                nc.scalar.copy(qr_sb[:], psm[0:1, 32:36])
                qrb = psml.tile([128, RANK], f32, tag="qrb")
                nc.gpsimd.partition_broadcast(qrb[:], qr_sb[:])
                bias_r = psml.tile([1, G], f32, tag="biasr")
                nc.scalar.copy(bias_r[:], psm[0:1, 64:128])
                bias_bc = psml.tile([128, G], f32, tag="biasbc")
                nc.gpsimd.partition_broadcast(bias_bc[:], bias_r[:])

                lrt = psml.tile([128, NCH, RANK], f32, tag="lrt")
                nc.vector.tensor_tensor(lrt[:], kp_sb[:],
                                        qrb[:, None, :].to_broadcast((128, NCH, RANK)), AO.mult)
                lr = psml.tile([128, NCH], f32, tag="lr")
                nc.vector.reduce_sum(lr[:], lrt[:], axis=mybir.AxisListType.X)

                att = psml.tile([128, NCH + 1], f32, tag="att")
                pskv = psk[:].rearrange("p (c two) -> p c two", two=2)
                bbv = bias_bc[:].rearrange("p (c two) -> p c two", two=2)
                nc.vector.tensor_tensor(att[0:64, 0:NCH], pskv[0:64, :, 0], lr[0:64, :], AO.add)
                nc.vector.tensor_tensor(att[0:64, 0:NCH], att[0:64, 0:NCH], bbv[0:64, :, 0], AO.add)
                nc.vector.tensor_tensor(att[64:128, 0:NCH], pskv[64:128, :, 1], lr[64:128, :], AO.add)
                nc.vector.tensor_tensor(att[64:128, 0:NCH], att[64:128, 0:NCH], bbv[64:128, :, 1], AO.add)
                nc.vector.memset(att[:, NCH:NCH + 1], -1e9)
                nc.vector.tensor_copy(att[0:SFP, NCH:NCH + 1], psm[0:SFP, 0:1])

                # softmax over all 128 x 33 entries
                m1 = psml.tile([128, 1], f32, tag="m1")
                nc.vector.reduce_max(m1[:], att[:], axis=mybir.AxisListType.X)
                mg = psml.tile([128, 1], f32, tag="mg")
                nc.gpsimd.partition_all_reduce(mg[:], m1[:], 128, bass_isa.ReduceOp.max)
                negm = psml.tile([128, 1], f32, tag="negm")
                nc.vector.tensor_scalar(negm[:], mg[:], -1.0, None, AO.mult)
                e = psml.tile([128, NCH + 1], bf16, tag="e")
                ssum = psml.tile([128, 1], f32, tag="ssum")
                nc.scalar.activation(e[:], att[:], AF.Exp, bias=negm[:, 0:1], scale=1.0,
                                     alpha=0.0, accum_out=ssum[:])
                sg = psml.tile([128, 1], f32, tag="sg")
                nc.gpsimd.partition_all_reduce(sg[:], ssum[:], 128, bass_isa.ReduceOp.add)
                recip = psml.tile([128, 1], f32, tag="recip")
                nc.vector.reciprocal(recip[:], sg[:])

                # build lhsT cols: 0 = aw, 1-2 = aw*vs, (3-6 = vq already)
                ev = e[:, 0:NCH, None]
                nc.vector.tensor_scalar(aw3[:, :, 0:1], ev, recip[:, 0:1], None, AO.mult)
                nc.vector.scalar_tensor_tensor(aw3[:, :, 1:3], ev.to_broadcast((128, NCH, FD)),
                                               recip[:, 0:1], vsc[:], AO.mult, AO.mult)
                nc.vector.tensor_scalar(vt[:, :, 130:131], ev, recip[:, 0:1], None, AO.mult)
                awf = psml.tile([SFP, 1], f32, tag="awf")
                nc.vector.tensor_scalar(awf[:], e[0:SFP, NCH:NCH + 1], recip[0:SFP, 0:1],
                                        None, AO.mult)

                psv = psB.tile([7, 131], f32, tag="psv")
                for c in range(NCH):
                    nc.tensor.matmul(psv[:], aw3[:, c, :], vt[:, c, :],
                                     start=(c == 0), stop=(c == NCH - 1))

                # mn scalars at partition 0; broadcast to partitions 1,2
                mn2 = psml.tile([3, FD], f32, tag="mn2")
                nc.scalar.copy(mn2[0:1, :], psv[0:1, 128:130])
                mn2b = psml.tile([3, FD], f32, tag="mn2b")
                nc.gpsimd.partition_broadcast(mn2b[:], mn2[0:1, :], channels=3)
                stage = psml.tile([3, 128], f32, tag="stage")
                nc.vector.tensor_scalar(stage[0:3, 0:64], psv[0:3, 0:64], mn2b[0:3, 0:1],
                                        None, AO.add)
                nc.vector.tensor_scalar(stage[0:3, 64:128], psv[0:3, 64:128], mn2b[0:3, 1:2],
                                        None, AO.add)
                nc.sync.dma_start(out=rows_sb[idx:idx + 1, 0:64], in_=stage[1:2, 0:64])
                nc.sync.dma_start(out=rows_sb[idx:idx + 1, 64:128], in_=stage[2:3, 64:128])

                vr_sb = psml.tile([7, 1], f32, tag="vr")
                nc.scalar.copy(vr_sb[:], psv[:, 130:131])
                nc.tensor.matmul(woin_ps[:, idx:idx + 1], vpT_sb[:], vr_sb[:],
                                 start=True, stop=False)
                nc.tensor.matmul(woin_ps[:, idx:idx + 1], vf_sb[:], awf[:],
                                 start=False, stop=True)

        # ---- tail: transpose rows, combine, wo matmul ----
        ictx.close()
        psO = ctx.enter_context(tc.tile_pool(name="psO", bufs=1, space="PSUM"))
        trp = psC.tile([128, 16], f32, tag="misc")
        nc.tensor.transpose(trp[:], rows_sb[:], id16[:])
        tr_sb = const.tile([128, 16], f32)
        nc.scalar.copy(tr_sb[:], trp[:])
        woin_sb = const.tile([128, 16], f32)
        nc.vector.tensor_tensor(woin_sb[:], tr_sb[:], woin_ps[:], AO.add)

        wo_sb = const.tile([128, HPC, HID], f32)
        nc.sync.dma_start(out=wo_sb[:], in_=woT[:].rearrange("(c p) n -> p c n", p=128))
        for half in range(2):
            po = psO.tile([B, HID // 2], f32, tag="po")
            for h in range(HPC):
                for nb in range(4):
                    j0 = half * 2048 + nb * 512
                    nc.tensor.matmul(po[:, nb * 512:(nb + 1) * 512],
                                     woin_sb[:, h * B:(h + 1) * B], wo_sb[:, h, j0:j0 + 512],
                                     start=(h == 0), stop=(h == HPC - 1))
            osb = const.tile([B, HID // 2], f32, tag=f"osb{half}")
            nc.scalar.copy(osb[:], po[:])
            nc.sync.dma_start(out=out[:, half * 2048:(half + 1) * 2048], in_=osb[:])

    nc.compile()
    return nc


def _host_prep(inputs):
    hs = np.asarray(inputs["hidden_states"], np.float32)
    pos = np.asarray(inputs["position_ids"])
    inv = 1.0 / (THETA ** (np.arange(0, D, 2, dtype=np.float32) / D))
    fr = pos[:, 0].astype(np.float32)[:, None] * inv[None, :]
    emb = np.concatenate([fr, fr], axis=1)
    cos_b = np.cos(emb).astype(np.float32)
    sin_b = np.sin(emb).astype(np.float32)
    cost = np.ascontiguousarray(np.tile(cos_b, (1, HPC)))
    sint = np.ascontiguousarray(np.tile(sin_b, (1, HPC)))
    hidT = np.ascontiguousarray(hs[:, 0, :].T)

    wq, wk, wv, wo = (np.asarray(inputs[k], np.float32) for k in ("wq", "wk", "wv", "wo"))
    in_maps = []
    for core in range(NCORES):
        h0 = core * HPC
        sl = slice(h0 * D, (h0 + HPC) * D)
        hsl = slice(h0, h0 + HPC)

        def rearr(x):  # [B,HPC,SQ,w] -> [B,HPC,128,NCH,w]
            w = x.shape[-1]
            return np.ascontiguousarray(
                x.reshape(B, HPC, NCH, 128, w).transpose(0, 1, 3, 2, 4))

        vp = np.asarray(inputs["value_p"], np.float32)[:, hsl]  # [B,HPC,D,R]
        vpT = np.zeros((B, HPC, 7, D), np.float32)
        vpT[:, :, 3:7, :] = vp.transpose(0, 1, 3, 2)
        m = {
            "hidT": hidT, "cost": cost, "sint": sint,
            "wqT": np.ascontiguousarray(wq[sl].T),
            "wkT": np.ascontiguousarray(wk[sl].T),
            "wvT": np.ascontiguousarray(wv[sl].T),
            "woT": np.ascontiguousarray(wo[:, sl].T),
            "kcode": np.ascontiguousarray(np.asarray(inputs["k_quant"], np.int32)[:, hsl]),
            "kscale": np.ascontiguousarray(np.asarray(inputs["k_scale"], np.float32)[:, hsl]),
            "kmn": np.ascontiguousarray(np.asarray(inputs["k_mn"], np.float32)[:, hsl]),
            "kfT": np.ascontiguousarray(
                np.asarray(inputs["k_full"], np.float32)[:, hsl].transpose(0, 1, 3, 2)),
            "kp": rearr(np.asarray(inputs["key_p"], np.float32)[:, hsl]),
            "keyq": np.ascontiguousarray(np.asarray(inputs["key_q"], np.float32)[:, hsl]),
            "vcode": np.ascontiguousarray(np.asarray(inputs["v_quant"], np.int32)[:, hsl]),
            "vscT": rearr(np.asarray(inputs["v_scale"], np.float32)[:, hsl]),
            "vmnT": rearr(np.asarray(inputs["v_mn"], np.float32)[:, hsl]),
            "vqT": rearr(np.asarray(inputs["value_q"], np.float32)[:, hsl]),
            "vpT": vpT,
            "vfr": np.ascontiguousarray(np.asarray(inputs["v_full"], np.float32)[:, hsl]),
        }
        in_maps.append(m)
    return in_maps


def kernel(**inputs):
    if "nc" not in _CACHE:
        _CACHE["nc"] = _build()
    nc = _CACHE["nc"]
    in_maps = _host_prep(inputs)
    res = run_bass_kernel_spmd(nc, in_maps, list(range(NCORES)),
                               trace=bool(os.environ.get("K_TRACE")))
    kernel.last = res
    total = np.zeros((B, HID), np.float32)
    for r in res.results:
        total += r["out"]
    return total.reshape(B, QL, HID)



# revision 13
# speedup vs baseline: 2.7194x; 2.7194x over previous
"""GEAR quantized-KV Llama attention decode step on 8 trn2 NeuronCores.

Sharding: tensor-parallel over heads (4 heads/core x 8 cores), all batches on
every core; each core computes a partial wo-product, summed on host.

v2: fp8 quant codes (no cast-DMA, HWDGE), fp8 DoubleRow matmuls for
projections and value side, batched small-tensor DMAs, max-free softmax,
column-oriented V output (no SBUF row DMAs), tail-only wo matmul.
"""
import os
import sys
import math

sys.path.insert(0, "/opt/trn_rl_repo")
import numpy as np
from contextlib import ExitStack

import concourse.bass as bass
import concourse.mybir as mybir
import concourse.tile as tile
from concourse import bacc, bass_isa
from concourse.bass_utils import run_bass_kernel_spmd
from concourse.masks import make_identity

B, H, D, HID = 4, 32, 128, 4096
SQ, SF, QL = 4096, 63, 1
GS, RANK = 64, 4
THETA = 10000.0
NCORES = 8
HPC = H // NCORES          # heads per core = 4
NP = B * HPC               # (b,h) pairs per core = 16
NCH = SQ // 128            # 32 s-chunks
G = SQ // GS               # 64 groups along seq (K side)
FD = D // GS               # 2 groups along head_dim (V side)
SFP = SF + 1               # 64 full-precision keys incl the new token
DT = mybir.dt
ISQD = 1.0 / math.sqrt(D)
WS = 16.0                  # fp8 weight pre-scale
QS = 256.0                 # fp8 qs pre-scale
VS = 16.0                  # fp8 aw*vscale pre-scale
VTW = 132                  # padded vt row width (128 codes + e-col + pad)
BLOBW = 64 + 68 + NCH * RANK + NCH * FD  # 324

_CACHE = {}


def _build():
    nc = bacc.Bacc("TRN2", target_bir_lowering=False)
    f32, bf16, fp8 = DT.float32, DT.bfloat16, DT.float8e4

    # ---- DRAM parameters (per core) ----
    hid8 = nc.declare_dram_parameter("hid8", [128, 32, B], fp8, isOutput=False)
    w8 = {w: nc.declare_dram_parameter(w, [128, 32, HPC * D], fp8, isOutput=False)
          for w in ("wq8", "wk8", "wv8")}
    woT = nc.declare_dram_parameter("woT", [128, HPC, HID], bf16, isOutput=False)
    ropeM = nc.declare_dram_parameter("ropeM", [128, B, 128], bf16, isOutput=False)
    blob = nc.declare_dram_parameter("blob", [128, NP, BLOBW], bf16, isOutput=False)
    vpmn = nc.declare_dram_parameter("vpmn", [8, NP, 128], bf16, isOutput=False)
    kfull = nc.declare_dram_parameter("kfull", [128, NP, SFP], bf16, isOutput=False)
    vfull = nc.declare_dram_parameter("vfull", [SFP, NP, 128], bf16, isOutput=False)
    kc8 = nc.declare_dram_parameter("kc8", [B, HPC, 128, SQ], fp8, isOutput=False)
    vt8 = nc.declare_dram_parameter("vt8", [B, HPC, 128, NCH, VTW], fp8, isOutput=False)
    vqmn = nc.declare_dram_parameter("vqmn", [B, HPC, 128, NCH, 16], fp8, isOutput=False)
    out = nc.declare_dram_parameter("out", [B, HID], f32, isOutput=True)

    AO = mybir.AluOpType
    AF = mybir.ActivationFunctionType
    PM = mybir.MatmulPerfMode

    with tile.TileContext(nc) as tc, ExitStack() as ctx:
        const = ctx.enter_context(tc.tile_pool(name="const", bufs=1))
        pw = ctx.enter_context(tc.tile_pool(name="pw", bufs=2))
        pctx = ctx.enter_context(ExitStack())
        psP = pctx.enter_context(tc.tile_pool(name="psP", bufs=1, space="PSUM"))
        psR = pctx.enter_context(tc.tile_pool(name="psR", bufs=2, space="PSUM"))

        # ---- constants / upfront loads ----
        id4 = const.tile([4, 4], f32)
        make_identity(nc, id4[:])
        hid_sb = const.tile([128, 32, B], fp8)
        nc.sync.dma_start(out=hid_sb[:], in_=hid8[:])
        ropeM_sb = const.tile([128, B, 128], bf16)
        nc.sync.dma_start(out=ropeM_sb[:], in_=ropeM[:])
        blob_sb = const.tile([128, NP, BLOBW], bf16)
        nc.scalar.dma_start(out=blob_sb[:], in_=blob[:])
        vpmn_sb = const.tile([8, NP, 128], bf16)
        nc.scalar.dma_start(out=vpmn_sb[:], in_=vpmn[:])
        kfull_sb = const.tile([128, NP, SFP], bf16)
        nc.scalar.dma_start(out=kfull_sb[:], in_=kfull[:])
        vfull_sb = const.tile([SFP, NP, 128], bf16)
        nc.scalar.dma_start(out=vfull_sb[:], in_=vfull[:])

        # ---- projections (fp8 DoubleRow): pps[b, 512] = hid @ w.T * WS ----
        proj_ps = {}
        for wname in ("wq8", "wk8", "wv8"):
            slab = pw.tile([128, 32, HPC * D], fp8, tag="wslab")
            nc.sync.dma_start(out=slab[:], in_=w8[wname][:])
            pps = psP.tile([B, HPC * D], f32, tag="proj" + wname)
            for kk in range(32):
                nc.tensor.matmul(pps[:], hid_sb[:, kk, :], slab[:, kk, :],
                                 start=(kk == 0), stop=(kk == 31))
            proj_ps[wname] = pps

        # v in row layout (for the new-token value row)
        v_sb = const.tile([B, HPC * D], bf16)
        nc.scalar.mul(v_sb[:], proj_ps["wv8"][:], 1.0 / WS)

        # ---- q/k: psum rows -> sbuf -> per-head transpose -> RoPE matmul ----
        qscT = const.tile([128, NP], bf16)   # cols idx = h*B+b, scaled 1/sqrt(D)
        kT = const.tile([128, NP], bf16)
        for wname, dst, scale in (("wq8", qscT, ISQD / WS), ("wk8", kT, 1.0 / WS)):
            row_sb = const.tile([B, HPC * D], f32, tag="row" + wname)
            nc.scalar.copy(row_sb[:], proj_ps[wname][:])
            colT = const.tile([128, HPC, B], bf16, tag="colT" + wname)
            for h in range(HPC):
                pt = psR.tile([128, B], f32, tag="tmp")
                nc.tensor.transpose(pt[:], row_sb[0:B, h * D:(h + 1) * D], id4[:])
                nc.scalar.copy(colT[:, h, :], pt[:])
            dstv = dst[:].rearrange("p (h b) -> p h b", b=B)
            for b in range(B):
                ro = psR.tile([128, HPC], f32, tag="tmp")
                nc.tensor.matmul(ro[:], ropeM_sb[:, b, :], colT[:, :, b],
                                 start=True, stop=True)
                nc.scalar.mul(dstv[:, :, b], ro[:], scale)

        # new-token k/v into resident full-precision tiles (one DMA each)
        nc.sync.dma_start(out=kfull_sb[:, :, SF:SFP], in_=kT[:])
        nc.scalar.dma_start(out=vfull_sb[SF:SFP, :, :], in_=v_sb[:])

        pctx.close()
        ictx = ctx.enter_context(ExitStack())
        pkc = ictx.enter_context(tc.tile_pool(name="pkc", bufs=3))
        pvt = ictx.enter_context(tc.tile_pool(name="pvt", bufs=3))
        paw = ictx.enter_context(tc.tile_pool(name="paw", bufs=3))
        psml = ictx.enter_context(tc.tile_pool(name="psml", bufs=3))
        psK = ictx.enter_context(tc.tile_pool(name="psK", bufs=2, space="PSUM"))
        psV = ictx.enter_context(tc.tile_pool(name="psV", bufs=2, space="PSUM"))
        psM = ictx.enter_context(tc.tile_pool(name="psM", bufs=2, space="PSUM"))
        psW = ictx.enter_context(tc.tile_pool(name="psW", bufs=2, space="PSUM"))

        woin_sb = const.tile([128, NP], bf16)

        # ---- per (b, h) attention ----
        for h in range(HPC):
            for b in range(B):
                idx = h * B + b
                vidx = b * HPC + h
                qcol = qscT[:, idx:idx + 1]

                kc = pkc.tile([128, SQ], fp8, tag="kc")
                nc.sync.dma_start(out=kc[:], in_=kc8[b, h])
                vt = pvt.tile([128, NCH, VTW], fp8, tag="vt")
                nc.sync.dma_start(out=vt[:], in_=vt8[b, h])
                aw3 = paw.tile([128, NCH, 16], fp8, tag="aw3")
                nc.scalar.dma_start(out=aw3[:], in_=vqmn[b, h])

                # qs8[d, g] = q_d * kscale[d,g] * 256  (fp8; x256 folded on host)
                qs8 = psml.tile([128, G], fp8, tag="qs8")
                nc.vector.tensor_tensor(qs8[:], blob_sb[:, idx, 0:64],
                                        qcol.to_broadcast((128, G)), AO.mult)

                # quant K scores: psk[s, 2c+g'] = sum_d code*qs8
                psk = psK.tile([128, 2 * NCH], f32, tag="psk")
                for c in range(NCH):
                    nc.tensor.matmul(psk[:, 2 * c:2 * c + 2], kc[:, c * 128:(c + 1) * 128],
                                     qs8[:, 2 * c:2 * c + 2], start=True, stop=True)
                psm = psM.tile([128, 80], f32, tag="psm")
                # kf scores incl new token -> psm[0:64, 0:1]
                nc.tensor.matmul(psm[0:SFP, 0:1], kfull_sb[:, idx, :], qcol,
                                 start=True, stop=True)
                # qr row + (x256) mn bias row -> psm[0:1, 4:72]
                nc.tensor.matmul(psm[0:1, 4:72], qcol, blob_sb[:, idx, 64:132],
                                 start=True, stop=True)
                qrmn_sb = psml.tile([1, 68], bf16, tag="qrmn")
                nc.scalar.copy(qrmn_sb[:], psm[0:1, 4:72])
                qrb = psml.tile([128, 68], bf16, tag="qrb")
                nc.gpsimd.partition_broadcast(qrb[:], qrmn_sb[:])

                # low-rank correction lr[s, c]
                kpv = blob_sb[:, idx, 132:132 + NCH * RANK].rearrange(
                    "p (c r) -> p c r", r=RANK)
                lrt = psml.tile([128, NCH, RANK], f32, tag="lrt")
                nc.vector.tensor_tensor(lrt[:], kpv,
                                        qrb[:, None, 0:4].to_broadcast((128, NCH, RANK)),
                                        AO.mult)
                lr = psml.tile([128, NCH], f32, tag="lr")
                nc.vector.reduce_sum(lr[:], lrt[:], axis=mybir.AxisListType.X)

                # att = psk/256 + lr + mn-bias  (+ kf col)
                att = psml.tile([128, NCH + 1], f32, tag="att")
                pskv = psk[:].rearrange("p (c two) -> p c two", two=2)
                bbv = qrb[:, 4:68].rearrange("p (c two) -> p c two", two=2)
                nc.vector.scalar_tensor_tensor(att[0:64, 0:NCH], pskv[0:64, :, 0],
                                               1.0 / QS, lr[0:64, :], AO.mult, AO.add)
                nc.vector.scalar_tensor_tensor(att[64:128, 0:NCH], pskv[64:128, :, 1],
                                               1.0 / QS, lr[64:128, :], AO.mult, AO.add)
                nc.vector.tensor_tensor(att[0:64, 0:NCH], att[0:64, 0:NCH],
                                        bbv[0:64, :, 0], AO.add)
                nc.vector.tensor_tensor(att[64:128, 0:NCH], att[64:128, 0:NCH],
                                        bbv[64:128, :, 1], AO.add)
                nc.vector.tensor_copy(att[0:SFP, NCH:NCH + 1], psm[0:SFP, 0:1])
                nc.vector.memset(att[SFP:128, NCH:NCH + 1], -1e30)

                # unnormalized softmax: e = exp(att), global sum
                e = psml.tile([128, NCH + 1], bf16, tag="e")
                ssum = psml.tile([128, 1], f32, tag="ssum")
                nc.scalar.activation(e[:], att[:], AF.Exp, accum_out=ssum[:])
                sg = psml.tile([128, 1], f32, tag="sg")
                nc.gpsimd.partition_all_reduce(sg[:], ssum[:], 128, bass_isa.ReduceOp.add)
                recip = psml.tile([128, 1], f32, tag="recip")
                nc.vector.reciprocal(recip[:], sg[:])
                recipV = psml.tile([128, 1], f32, tag="recipV")
                nc.vector.tensor_scalar(recipV[:], recip[:], 1.0 / VS, None, AO.mult)

                # fp8 operands for V side
                vscv = blob_sb[:, idx, 260:324].rearrange("p (c two) -> p c two", two=2)
                nc.vector.scalar_tensor_tensor(
                    aw3[:, :, 0:2], e[:, 0:NCH, None].to_broadcast((128, NCH, 2)),
                    VS, vscv, AO.mult, AO.mult)
                nc.vector.tensor_copy(vt[:, :, 128:129], e[:, 0:NCH, None])

                # V matmuls (DoubleRow over chunk pairs): psv[8, VTW]
                psv = psV.tile([16, VTW], f32, tag="psv")
                for c in range(NCH // 2):
                    nc.tensor.matmul(psv[:], aw3[:, 2 * c:2 * c + 2, :],
                                     vt[:, 2 * c:2 * c + 2, :],
                                     start=(c == 0), stop=(c == NCH // 2 - 1),
                                     perf_mode=PM.DoubleRow)

                # full-precision V + low-rank + mn into its own psum column
                psw = psW.tile([128, 1], f32, tag="psw")
                awf = psml.tile([SFP, 1], bf16, tag="awf")
                nc.vector.tensor_scalar(awf[:], e[0:SFP, NCH:NCH + 1],
                                        recip[0:SFP, 0:1], None, AO.mult)
                nc.tensor.matmul(psw[:], vfull_sb[:, vidx, :], awf[:],
                                 start=True, stop=False)
                colsb = psml.tile([8, 1], bf16, tag="colsb")
                nc.vector.tensor_scalar(colsb[:], psv[0:8, 128:129],
                                        recip[0:8, 0:1], None, AO.mult)
                nc.tensor.matmul(psw[:], vpmn_sb[:, idx, :], colsb[:],
                                 start=False, stop=True)

                # quant V halves: transpose rows 0:2 -> columns, combine
                vT = psml.tile([2, 128], f32, tag="vT")
                nc.scalar.copy(vT[:], psv[0:2, 0:128])
                nc.tensor.transpose(psm[:, 74:76], vT[:], id4[0:2, 0:2])
                wsb = psml.tile([128, 1], f32, tag="wsb")
                nc.scalar.copy(wsb[:], psw[:])
                nc.vector.scalar_tensor_tensor(woin_sb[0:64, idx:idx + 1],
                                               psm[0:64, 74:75], recipV[0:64, 0:1],
                                               wsb[0:64, :], AO.mult, AO.add)
                nc.vector.scalar_tensor_tensor(woin_sb[64:128, idx:idx + 1],
                                               psm[64:128, 75:76], recipV[64:128, 0:1],
                                               wsb[64:128, :], AO.mult, AO.add)

        # ---- tail: wo matmul ----
        ictx.close()
        psO = ctx.enter_context(tc.tile_pool(name="psO", bufs=2, space="PSUM"))
        wo_sb = const.tile([128, HPC, HID], bf16)
        nc.sync.dma_start(out=wo_sb[:], in_=woT[:])
        for half in range(2):
            po = psO.tile([B, HID // 2], f32, tag="po")
            for h in range(HPC):
                for nb in range(4):
                    j0 = half * 2048 + nb * 512
                    nc.tensor.matmul(po[:, nb * 512:(nb + 1) * 512],
                                     woin_sb[:, h * B:(h + 1) * B],
                                     wo_sb[:, h, j0:j0 + 512],
                                     start=(h == 0), stop=(h == HPC - 1))
            osb = const.tile([B, HID // 2], f32, tag=f"osb{half}")
            nc.scalar.copy(osb[:], po[:])
            nc.sync.dma_start(out=out[:, half * 2048:(half + 1) * 2048], in_=osb[:])

    nc.compile()
    return nc


def _host_prep(inputs):
    f8 = mybir.dt.np(mybir.dt.float8e4)
    bf = mybir.dt.np(mybir.dt.bfloat16)
    hs = np.asarray(inputs["hidden_states"], np.float32)
    pos = np.asarray(inputs["position_ids"])

    # rope matrices M_b^T (bf16): q_roped = M_b @ q
    inv = 1.0 / (THETA ** (np.arange(0, D, 2, dtype=np.float32) / D))
    fr = pos[:, 0].astype(np.float32)[:, None] * inv[None, :]
    emb = np.concatenate([fr, fr], axis=1)          # [B, 128]
    cos_b, sin_b = np.cos(emb), np.sin(emb)
    M = np.zeros((B, D, D), np.float32)
    dd = np.arange(D)
    M[:, dd, dd] = cos_b
    M[:, dd[:64], dd[:64] + 64] = -sin_b[:, :64]
    M[:, dd[64:], dd[64:] - 64] = sin_b[:, 64:]
    ropeM = np.ascontiguousarray(
        M.transpose(2, 0, 1)).astype(bf)             # [128(k), B, 128(m)]

    # hid chunk-paired fp8: [128, 16, 2, B]
    hidT = hs[:, 0, :].T                             # [HID, B]
    hid8 = np.ascontiguousarray(
        hidT.reshape(32, 128, B).transpose(1, 0, 2)).astype(f8)

    wq, wk, wv, wo = (np.asarray(inputs[k], np.float32) for k in ("wq", "wk", "wv", "wo"))
    kq_all = np.asarray(inputs["k_quant"], np.int32)
    ks_all = np.asarray(inputs["k_scale"], np.float32)
    km_all = np.asarray(inputs["k_mn"], np.float32)
    kf_all = np.asarray(inputs["k_full"], np.float32)
    kp_all = np.asarray(inputs["key_p"], np.float32)
    keyq_all = np.asarray(inputs["key_q"], np.float32)
    vq_all = np.asarray(inputs["v_quant"], np.int32)
    vs_all = np.asarray(inputs["v_scale"], np.float32)
    vm_all = np.asarray(inputs["v_mn"], np.float32)
    vf_all = np.asarray(inputs["v_full"], np.float32)
    vvq_all = np.asarray(inputs["value_q"], np.float32)
    vvp_all = np.asarray(inputs["value_p"], np.float32)

    in_maps = []
    for core in range(NCORES):
        h0 = core * HPC
        sl = slice(h0 * D, (h0 + HPC) * D)
        hsl = slice(h0, h0 + HPC)

        def wslab(w):   # [512, HID] -> [128, 16, 2, 512] fp8 (x WS)
            wT = w[sl].T * WS                        # [HID, 512]
            return np.ascontiguousarray(
                wT.reshape(32, 128, HPC * D).transpose(1, 0, 2)).astype(f8)

        # blob [128, NP, 324]: ksc | keyq + kmn*QS | kp | vsc
        blobc = np.zeros((128, NP, BLOBW), np.float32)
        kfullc = np.zeros((128, NP, SFP), np.float32)
        vfullc = np.zeros((SFP, NP, 128), np.float32)
        vpmnc = np.zeros((8, NP, 128), np.float32)
        for h in range(HPC):
            for b in range(B):
                idx = h * B + b
                vidx = b * HPC + h
                blobc[:, idx, 0:64] = ks_all[b, h0 + h] * QS     # [128, 64]
                blobc[:, idx, 64:68] = keyq_all[b, h0 + h]       # [128, 4]
                blobc[:, idx, 68:132] = km_all[b, h0 + h]        # [128, 64]
                kp = kp_all[b, h0 + h].reshape(NCH, 128, RANK)   # [c, sp, r]
                blobc[:, idx, 132:132 + NCH * RANK] = \
                    kp.transpose(1, 0, 2).reshape(128, NCH * RANK)
                vs = vs_all[b, h0 + h].reshape(NCH, 128, FD)
                blobc[:, idx, 260:324] = vs.transpose(1, 0, 2).reshape(128, NCH * FD)
                kfullc[:, idx, 0:SF] = kf_all[b, h0 + h].T       # [128, 63]
                vfullc[0:SF, vidx, :] = vf_all[b, h0 + h]        # [63, 128]
                vpmnc[2:6, idx, :] = vvp_all[b, h0 + h].T        # value_p^T
                vpmnc[6, idx, 0:64] = 1.0
                vpmnc[7, idx, 64:128] = 1.0

        # vt blob [B, HPC, 128, NCH, VTW] fp8: codes + zero cols
        vq = vq_all[:, hsl].reshape(B, HPC, NCH, 128, D)
        vtc = np.zeros((B, HPC, 128, NCH, VTW), f8)
        vtc[:, :, :, :, 0:128] = vq.transpose(0, 1, 3, 2, 4).astype(f8)

        # vqmn [B, HPC, 128, NCH, 8] fp8: cols 0:2 zero, 2:6 vq, 6:8 vmn
        vvq = vvq_all[:, hsl].reshape(B, HPC, NCH, 128, RANK)
        vm = vm_all[:, hsl].reshape(B, HPC, NCH, 128, FD)
        vqmnc = np.zeros((B, HPC, 128, NCH, 16), f8)
        vqmnc[:, :, :, :, 2:6] = vvq.transpose(0, 1, 3, 2, 4).astype(f8)
        vqmnc[:, :, :, :, 6:8] = vm.transpose(0, 1, 3, 2, 4).astype(f8)

        m = {
            "hid8": hid8, "ropeM": ropeM,
            "wq8": wslab(wq), "wk8": wslab(wk), "wv8": wslab(wv),
            "woT": np.ascontiguousarray(
                wo[:, sl].T.reshape(HPC, 128, HID).transpose(1, 0, 2)).astype(bf),
            "blob": blobc.astype(bf),
            "vpmn": vpmnc.astype(bf),
            "kfull": kfullc.astype(bf),
            "vfull": vfullc.astype(bf),
            "kc8": np.ascontiguousarray(kq_all[:, hsl]).astype(f8),
            "vt8": vtc,
            "vqmn": vqmnc,
        }
        in_maps.append(m)
    return in_maps


def kernel(**inputs):
    if "nc" not in _CACHE:
        _CACHE["nc"] = _build()
    nc = _CACHE["nc"]
    in_maps = _host_prep(inputs)
    res = run_bass_kernel_spmd(nc, in_maps, list(range(NCORES)),
                               trace=bool(os.environ.get("K_TRACE")))
    kernel.last = res
    total = np.zeros((B, HID), np.float32)
    for r in res.results:
        total += r["out"]
    return total.reshape(B, QL, HID)
